# revision 1
# baseline (speedup 1.0000x reference)
"""CoAttention kernel for Trainium2, data-parallel over batch across 8 NeuronCores.

Per core (one batch element b):
    query = data1[b] @ Wq + bq                      # [2048, 256]
    key   = data2[b] @ Wk + bk                      # [2048, 256]
    attn  = softmax(SCALE * query @ key^T)          # row-constant terms cancel
    out   = attn @ key + query

Device-side strategy (v4):
  - All input DMAs issue up-front: activations on the sync HWDGE ring
    (d1g0, d2h0, d1g1, d2h1, d1g2, d1g3), weights + biases on the scalar
    HWDGE ring in parallel.  fp32 -> bf16 casts run on DVE (activations)
    and ACT (weights, idle pre-exp).
  - softmax(q@(k+bk)^T) drops bias terms constant along k, and
    sum(attn)==1 makes attn@(key+bk) == attn@key + bk, so the key value
    matrix carries NO bias; bq biases the scores path and (bq+bk) the
    residual path from the same QT PSUM.  The softmax denominator is a
    memset 1.0 column appended to the fp8 key values.
  - Transposes: early ones (d1 g0/g1, d2 h0) run on the PE while it is
    otherwise idle; mid-kernel ones (d1 g2/g3, d2 h1, Q-residual) run as
    batched xbar DMA transposes ([128, n*128] -> [128, n, 128]) on the
    sync ring, freeing the PE during the exp-bound phase.
  - scoresT [k, q] orientation lets exp(scoresT) feed the context matmul
    as the stationary operand; scores and context run in fp8e4m3
    DoubleRow.  ctx for the second q-half is split kp0-5 (runs inside the
    exp stream, evicted to bf16) + kp6-7 (after the last exp) to shrink
    the serial tail.
  - ACT does only the 32 serial exps plus small pre-exp work; GPSIMD never
    touches PSUM.  24 warmup matmuls ramp the PE p-state early.
  - Output is written in 8 chunks of 256 rows as each completes.
"""

import sys

if "/opt/trn_rl_repo" not in sys.path:
    sys.path.insert(0, "/opt/trn_rl_repo")

from contextlib import ExitStack

import numpy as np

import concourse.bass as bass  # noqa: F401
import concourse.mybir as mybir
import concourse.tile as tile
from concourse import bacc
from concourse.bass_utils import run_bass_kernel_spmd
from concourse.masks import make_identity

B, LQ, LK, DIN, D = 8, 2048, 2048, 1024, 256
N_CORES = 8
SCALE = float(1.0 / np.sqrt(1024.0).astype(np.float32))

BF16 = mybir.dt.bfloat16
FP8 = mybir.dt.float8e4
F32 = mybir.dt.float32
AF = mybir.ActivationFunctionType
PM_DR = mybir.MatmulPerfMode.DoubleRow
ADD = mybir.AluOpType.add
MULT = mybir.AluOpType.mult


def _build():
    nc = bacc.Bacc("TRN2", target_bir_lowering=False, debug=False)
    d1 = nc.dram_tensor("data1", [LQ, DIN], F32, kind="ExternalInput").ap()
    d2 = nc.dram_tensor("data2", [LK, D], F32, kind="ExternalInput").ap()
    wpk = nc.dram_tensor("wpack", [128, 2564], F32, kind="ExternalInput").ap()
    out = nc.dram_tensor("out", [LQ, D], F32, kind="ExternalOutput").ap()

    WPACK = 2564
    QB = LQ // 128  # 16 q blocks
    KB = LK // 128  # 16 k blocks
    IC1 = DIN // 128  # 8
    IC2 = D // 128  # 2
    KP = KB // 2  # 8 fp8 DoubleRow k-pairs

    with tile.TileContext(nc) as tc, ExitStack() as ctx:
        const = ctx.enter_context(tc.tile_pool(name="const", bufs=1))
        big = ctx.enter_context(tc.tile_pool(name="big", bufs=1))
        stage1 = ctx.enter_context(tc.tile_pool(name="stage1", bufs=3))
        stage2 = ctx.enter_context(tc.tile_pool(name="stage2", bufs=2))
        small = ctx.enter_context(tc.tile_pool(name="small", bufs=4))
        ps_gp = ctx.enter_context(tc.tile_pool(name="ps_gp", bufs=2, space="PSUM"))
        ps_tr = ctx.enter_context(tc.tile_pool(name="ps_tr", bufs=2, space="PSUM"))
        ps_sc = ctx.enter_context(tc.tile_pool(name="ps_sc", bufs=2, space="PSUM"))

        # ---------------- constants ----------------
        ident_bf = const.tile([128, 128], BF16, tag="ident_bf")
        make_identity(nc, ident_bf[:])
        warm_src = const.tile([128, 512], BF16, tag="warm_src")
        nc.gpsimd.memset(warm_src[:], 0.0)

        key2 = [
            big.tile([128, 2, D + 1], FP8, tag=f"key2_{kp}", name=f"key2_{kp}")
            for kp in range(KP)
        ]
        for kp in range(KP):
            nc.gpsimd.memset(key2[kp][:, :, D:D + 1], 1.0)

        # ---------------- loads -------------------------------------------
        # wpack = host-packed [wq | wk | bq | bk] in final SBUF layout: one
        # fast contiguous DMA first on the sync ring, then the d1 groups.
        # d2 halves ride the scalar ring (only 2 issue ops on the ACT queue).
        d2_st = [stage2.tile([128, 8 * D], F32, tag="d2st", name=f"d2st{hh}")
                 for hh in range(2)]
        d1_st = [stage1.tile([128, 4 * DIN], F32, tag="d1st", name=f"d1st{g}")
                 for g in range(4)]
        wpack_st = const.tile([128, WPACK], F32, tag="wpack")

        def load_d2(hh):
            nc.sync.dma_start(
                out=d2_st[hh][:].rearrange("p (t i) -> p t i", i=D),
                in_=d2[hh * 1024:(hh + 1) * 1024, :].rearrange(
                    "(t p) i -> p t i", p=128),
            )

        def load_d1(g):
            nc.sync.dma_start(
                out=d1_st[g][:].rearrange("p (t i) -> p t i", i=DIN),
                in_=d1[g * 512:(g + 1) * 512, :].rearrange("(t p) i -> p t i", p=128),
            )

        nc.sync.dma_start(out=wpack_st[:], in_=wpk)
        load_d1(0)
        load_d1(1)
        load_d2(0)
        load_d2(1)
        load_d1(2)
        load_d1(3)
        bq_col = wpack_st[:, 2560:2562]
        bk_col = wpack_st[:, 2562:2564]

        # ---------------- PE p-state warmup ---------------------------------
        for w in range(12):
            pw = ps_gp.tile([128, 512], F32, tag="ps_gp", name=f"warm{w}")
            nc.tensor.matmul(pw[:], lhsT=ident_bf[:], rhs=warm_src[:],
                             start=True, stop=True)

        # ---------------- weight cast (ACT) + residual bias (gpsimd) --------
        wqk_sb = const.tile([128, IC1 * D + IC2 * D], BF16, tag="wqk_sb")
        nc.scalar.copy(wqk_sb[:], wpack_st[:, :2560])
        wqs = [wqk_sb[:, i * D:(i + 1) * D] for i in range(IC1)]
        wks = [wqk_sb[:, 2048 + i * D:2048 + (i + 1) * D] for i in range(IC2)]
        bqk_col = const.tile([128, IC2], F32, tag="bqk_col")
        nc.gpsimd.tensor_add(bqk_col[:], bq_col[:], bk_col[:])

        # ---------------- transposed activations ----------------------------
        # d1T[:, ic, q] : d1[q, ic*128 + p];  d2T[:, ic, k] : d2[k, ic*128 + p]
        d1T = big.tile([128, IC1, LQ], BF16, tag="d1T")
        d2T = big.tile([128, IC2, LK], BF16, tag="d2T")
        d1_bf = [stage2.tile([128, 4 * DIN], BF16, tag="d1bf", name=f"d1bf{g}",
                             bufs=1)
                 for g in range(4)]
        d2_bf = [stage2.tile([128, 8 * D], BF16, tag="d2bf", name=f"d2bf{hh}",
                             bufs=1)
                 for hh in range(2)]

        def d1_cast(g, on_act=False):
            if on_act:
                nc.scalar.copy(d1_bf[g][:], d1_st[g][:])
            else:
                nc.vector.tensor_copy(d1_bf[g][:], d1_st[g][:])

        def d1_cast_ic(g, ic):
            src = d1_st[g][:].rearrange("p (t i) -> p t i", i=DIN)
            dst = d1_bf[g][:].rearrange("p (t i) -> p t i", i=DIN)
            nc.vector.tensor_copy(
                dst[:, :, ic * 128:(ic + 1) * 128],
                src[:, :, ic * 128:(ic + 1) * 128],
            )

        def d2_cast(hh):
            nc.vector.tensor_copy(d2_bf[hh][:], d2_st[hh][:])

        # PE transpose path (early groups, PE otherwise idle)
        def d1T_pe(g, ic):
            bf = d1_bf[g]
            pt = ps_tr.tile([128, 512], BF16, tag="ps_t", name=f"pt_d1_{g}_{ic}")
            for j in range(4):
                nc.tensor.transpose(
                    pt[:, j * 128:(j + 1) * 128],
                    bf[:, j * DIN + ic * 128: j * DIN + (ic + 1) * 128],
                    ident_bf[:],
                )
            nc.vector.tensor_copy(d1T[:, ic, g * 512:(g + 1) * 512], pt[:])

        def d2T_pe(hh, ic, h4):
            bf = d2_bf[hh]
            pt = ps_tr.tile([128, 512], BF16, tag="ps_t",
                            name=f"pt_d2_{hh}_{ic}_{h4}")
            for j in range(4):
                kt = 4 * h4 + j
                nc.tensor.transpose(
                    pt[:, j * 128:(j + 1) * 128],
                    bf[:, kt * D + ic * 128: kt * D + (ic + 1) * 128],
                    ident_bf[:],
                )
            nc.vector.tensor_copy(
                d2T[:, ic, hh * 1024 + h4 * 512: hh * 1024 + (h4 + 1) * 512],
                pt[:],
            )


        # ---------------- K^T fp8 DoubleRow layout [128, 2, k] --------------
        kt_sb = big.tile([128, 2, LK], FP8, tag="kt_sb")

        def kt_unit(dc, nk):
            ps = ps_gp.tile([128, 512], F32, tag="ps_gp")
            for ic in range(IC2):
                nc.tensor.matmul(
                    ps[:],
                    lhsT=wks[ic][:, dc * 128:(dc + 1) * 128],
                    rhs=d2T[:, ic, nk * 512:(nk + 1) * 512],
                    start=(ic == 0),
                    stop=(ic == IC2 - 1),
                )
            nc.vector.tensor_copy(kt_sb[:, dc, nk * 512:(nk + 1) * 512], ps[:])

        # ---------------- key value pairs (no bias) --------------------------
        def key_pair(kp):
            ps = ps_gp.tile([128, 512], F32, tag="ps_gp")
            for s in range(2):
                kb = kp * 2 + s
                p = ps[:, s * D:(s + 1) * D]
                for ic in range(IC2):
                    nc.tensor.matmul(
                        p,
                        lhsT=d2T[:, ic, kb * 128:(kb + 1) * 128],
                        rhs=wks[ic],
                        start=(ic == 0),
                        stop=(ic == IC2 - 1),
                    )
            nc.vector.tensor_copy(
                key2[kp][:, :, :D],
                ps[:].rearrange("p (s d) -> p s d", s=2),
            )

        # ---------------- QT projection ------------------------------------
        qt_sb = big.tile([128, 2, LQ], FP8, tag="qt_sb")
        qtbf = big.tile([128, 2, LQ], BF16, tag="qtbf")

        def qt_mms(dc, nq, ps):
            for ic in range(IC1):
                nc.tensor.matmul(
                    ps[:],
                    lhsT=wqs[ic][:, dc * 128:(dc + 1) * 128],
                    rhs=d1T[:, ic, nq * 512:(nq + 1) * 512],
                    start=(ic == 0),
                    stop=(ic == IC1 - 1),
                )

        def qt_bias_sc(ps, dc, nq, on_act):
            o = qt_sb[:, dc, nq * 512:(nq + 1) * 512]
            if on_act:
                nc.scalar.activation(o, ps[:], AF.Identity,
                                     bias=bq_col[:, dc:dc + 1])
            else:
                nc.vector.tensor_scalar(o, ps[:], bq_col[:, dc:dc + 1], None, ADD)

        def qt_bias_rs(ps, dc, nq, on_act):
            o = qtbf[:, dc, nq * 512:(nq + 1) * 512]
            if on_act:
                nc.scalar.activation(o, ps[:], AF.Identity,
                                     bias=bqk_col[:, dc:dc + 1])
            else:
                nc.vector.tensor_scalar(o, ps[:], bqk_col[:, dc:dc + 1], None, ADD)

        def qt_unit(dc, nq, on_act=False):
            ps = ps_gp.tile([128, 512], F32, tag="ps_gp")
            qt_mms(dc, nq, ps)
            qt_bias_sc(ps, dc, nq, on_act)
            qt_bias_rs(ps, dc, nq, on_act)

        # ---------------- residual Q via xbar DMA transpose ------------------
        # qres3[qg][q_low, j, dc, c] = Q[qg*512 + j*128 + q_low, dc*128 + c]
        qres3 = [big.tile([128, 4, 2, 128], BF16, tag=f"qres{qg}",
                          name=f"qres{qg}")
                 for qg in range(4)]

        def qres_pe(qg, dc):
            pt = ps_tr.tile([128, 512], BF16, tag="ps_t", name=f"pt_q_{qg}_{dc}")
            for j in range(4):
                qb = qg * 4 + j
                nc.tensor.transpose(
                    pt[:, j * 128:(j + 1) * 128],
                    qtbf[:, dc, qb * 128:(qb + 1) * 128],
                    ident_bf[:],
                )
            nc.vector.tensor_copy(
                qres3[qg][:, :, dc, :],
                pt[:].rearrange("p (j c) -> p j c", c=128),
            )

        # ---------------- scores + exp --------------------------------------
        expT = [
            [big.tile([128, 2, 1024], FP8, tag=f"expT{kp}_{nh}",
                      name=f"expT{kp}_{nh}")
             for nh in range(2)]
            for kp in range(KP)
        ]

        def scores_unit(km, nh):
            ps = ps_sc.tile([128, 1024], F32, tag="ps_sc")
            for half in range(2):
                nq = nh * 2 + half
                nc.tensor.matmul(
                    ps[:, half * 512:(half + 1) * 512],
                    lhsT=kt_sb[:, :, km * 128:(km + 1) * 128],
                    rhs=qt_sb[:, :, nq * 512:(nq + 1) * 512],
                    perf_mode=PM_DR,
                    start=True,
                    stop=True,
                )
            nc.scalar.activation(
                expT[km // 2][nh][:, km % 2, :], ps[:], AF.Exp, scale=SCALE
            )

        # ---------------- context + residual + out DMA ----------------------
        out_c = [stage2.tile([128, 2 * D], F32, tag="outc", name=f"outc{c}")
                 for c in range(QB // 2)]
        ctxA = [big.tile([128, D + 1], BF16, tag=f"ctxA{i}", name=f"ctxA{i}")
                for i in range(8)]
        KP_A = 6  # h1 ctx kp-split: A = kp0-5 inside exp stream, B = kp6-7 after

        def ctx_mm(pc, qb, kp, start, stop):
            h, qq = qb // 8, qb % 8
            nc.tensor.matmul(
                pc,
                lhsT=expT[kp][h][:, :, qq * 128:(qq + 1) * 128],
                rhs=key2[kp][:],
                perf_mode=PM_DR,
                start=start,
                stop=stop,
            )

        def ctx_finish(pc, qb):
            rc = small.tile([128, 1], F32, tag="recip")
            nc.vector.reciprocal(rc[:], pc[:, D:D + 1])
            c = qb // 2
            osl = out_c[c][:, (qb % 2) * D:(qb % 2 + 1) * D]
            nc.vector.tensor_scalar(osl, pc[:, :D], rc[:], None, MULT)
            qg, j = qb // 4, qb % 4
            nc.gpsimd.tensor_add(
                osl.rearrange("p (a b) -> p a b", a=2),
                osl.rearrange("p (a b) -> p a b", a=2),
                qres3[qg][:, j, :, :],
            )
            if qb % 2 == 1:
                nc.sync.dma_start(
                    out=out[c * 256:(c + 1) * 256, :].rearrange(
                        "(t p) d -> p t d", p=128),
                    in_=out_c[c][:].rearrange("p (t d) -> p t d", d=D),
                )

        def ctx_unit_h0(qb):
            pc_full = ps_gp.tile([128, 512], F32, tag="ps_gp")
            pc = pc_full[:, :D + 1]
            for kp in range(KP):
                ctx_mm(pc, qb, kp, kp == 0, kp == KP - 1)
            ctx_finish(pc, qb)

        def ctx_h1_A(qb):
            pc_full = ps_gp.tile([128, 512], F32, tag="ps_gp")
            pc = pc_full[:, :D + 1]
            for kp in range(KP_A):
                ctx_mm(pc, qb, kp, kp == 0, kp == KP_A - 1)
            nc.vector.tensor_copy(ctxA[qb - 8][:], pc)

        def ctx_h1_B(qb):
            pc_full = ps_sc.tile([128, 512], F32, tag="ps_sc")
            pc = pc_full[:, :D + 1]
            for kp in range(KP_A, KP):
                ctx_mm(pc, qb, kp, kp == KP_A, kp == KP - 1)
            nc.vector.tensor_tensor(pc, pc, ctxA[qb - 8][:], ADD)
            ctx_finish(pc, qb)

        # ================= emission schedule ================================
        def units(fn, idxs):
            return [lambda i=i: fn(*i) if isinstance(i, tuple) else fn(i)
                    for i in idxs]

        def interleave(a, b, ratio):
            a = list(a)
            b = list(b)
            ia = ib = 0
            credit = 0.0
            while ia < len(a) or ib < len(b):
                if ia < len(a):
                    a[ia]()
                    ia += 1
                credit += ratio
                while credit >= 1.0 and ib < len(b):
                    b[ib]()
                    ib += 1
                    credit -= 1.0
            while ib < len(b):
                b[ib]()
                ib += 1

        # --- phase 1a: d1 g0 -> PE transposes -> QT nq0 (d1 lands first) ---
        d1_cast(0, on_act=True)
        for ic in range(IC1):
            d1T_pe(0, ic)
        for dc in range(2):
            qt_unit(dc, 0, on_act=True)

        # --- phase 1b: d2 h0 -> PE transposes -> kt-h0 ---
        d2_cast(0)
        for ic in range(IC2):
            for h4 in range(2):
                d2T_pe(0, ic, h4)
        for nk in range(2):
            for dc in range(2):
                kt_unit(dc, nk)

        # --- phase 1c: d1 g1 per-ic pipelined -> QT nq1 (gates exp#0) ---
        ps_nq1 = []
        for dc in range(2):
            ps = ps_gp.tile([128, 512], F32, tag="ps_gp", name=f"qtps1_{dc}")
            for ic in range(IC1):
                if dc == 0:
                    d1_cast_ic(1, ic)
                    d1T_pe(1, ic)
                nc.tensor.matmul(
                    ps[:],
                    lhsT=wqs[ic][:, dc * 128:(dc + 1) * 128],
                    rhs=d1T[:, ic, 512:1024],
                    start=(ic == 0),
                    stop=(ic == IC1 - 1),
                )
            ps_nq1.append(ps)
        for dc in range(2):
            qt_bias_sc(ps_nq1[dc], dc, 1, on_act=True)
        for dc in range(2):
            qt_bias_rs(ps_nq1[dc], dc, 1, on_act=False)

        # residual transposes for q-half 0 (PE)
        for qg in range(2):
            for dc in range(2):
                qres_pe(qg, dc)

        # --- phase 2: scores-h0 interleaved with late loads + DMA T ---
        filler = (
            [lambda: d2_cast(1)]
            + units(d2T_pe, [(1, ic, h4) for ic in range(IC2)
                             for h4 in range(2)])
            + units(kt_unit, [(dc, nk) for nk in range(2, 4)
                              for dc in range(2)])
            + [lambda: d1_cast(2)]
            + units(d1T_pe, [(2, ic) for ic in range(IC1)])
            + units(qt_unit, [(0, 2), (1, 2)])
            + [lambda: d1_cast(3)]
            + units(d1T_pe, [(3, ic) for ic in range(IC1)])
            + units(qt_unit, [(0, 3), (1, 3)])
            + units(qres_pe, [(qg, dc) for qg in range(2, 4)
                              for dc in range(2)])
            + units(key_pair, list(range(8)))
        )
        scores_h0 = units(scores_unit, [(km, 0) for km in range(KB)])
        interleave(scores_h0, filler, len(filler) / len(scores_h0))

        # --- phase 3: scores-h1 with ctx-h0 and ctx-h1-A ---
        # ctx_h1_A reads exps km0-11 of h1, so A units may only be emitted
        # after scores-h1 km11 (Tile orders by emission-time dependencies)
        sc_h1_a = units(scores_unit, [(km, 1) for km in range(12)])
        sc_h1_b = units(scores_unit, [(km, 1) for km in range(12, KB)])
        ctx0 = units(ctx_unit_h0, list(range(0, 8)))
        ctxa = units(ctx_h1_A, list(range(8, 16)))
        interleave(sc_h1_a, ctx0, len(ctx0) / len(sc_h1_a))
        interleave(sc_h1_b, ctxa, len(ctxa) / len(sc_h1_b))

        # --- phase 4: ctx-h1-B tail ---
        for qb in range(8, 16):
            ctx_h1_B(qb)

    nc.compile()
    return nc


_NC = None
_last_in_maps = None


def _get_nc():
    global _NC
    if _NC is None:
        _NC = _build()
    return _NC


def make_wpack(Wq, Wk, bq, bk):
    Wq = np.asarray(Wq, dtype=np.float32)
    Wk = np.asarray(Wk, dtype=np.float32)
    bq = np.asarray(bq, dtype=np.float32)
    bk = np.asarray(bk, dtype=np.float32)
    wpack = np.empty((128, 2564), np.float32)
    for c in range(8):
        wpack[:, c * 256:(c + 1) * 256] = Wq[c * 128:(c + 1) * 128, :]
    for c in range(2):
        wpack[:, 2048 + c * 256:2048 + (c + 1) * 256] = Wk[c * 128:(c + 1) * 128, :]
    for c in range(2):
        wpack[:, 2560 + c] = bq[c * 128:(c + 1) * 128]
        wpack[:, 2562 + c] = bk[c * 128:(c + 1) * 128]
    return np.ascontiguousarray(wpack)


def kernel(data1, data2, Wq, bq, Wk, bk):
    global _last_in_maps
    data1 = np.asarray(data1, dtype=np.float32)
    data2 = np.asarray(data2, dtype=np.float32)
    wpack = make_wpack(Wq, Wk, bq, bk)

    nc = _get_nc()
    in_maps = [
        {
            "data1": np.ascontiguousarray(data1[b]),
            "data2": np.ascontiguousarray(data2[b]),
            "wpack": wpack,
        }
        for b in range(B)
    ]
    _last_in_maps = in_maps
    res = run_bass_kernel_spmd(nc, in_maps, core_ids=list(range(N_CORES)))
    return np.stack([res.results[i]["out"] for i in range(B)], axis=0)



# revision 2
# speedup vs baseline: 1.2714x; 1.2714x over previous
"""CoAttention kernel for Trainium2, data-parallel over batch across 8 NeuronCores.

Per core (one batch element b):
    query = data1[b] @ Wq + bq                      # [2048, 256]
    key   = data2[b] @ Wk + bk                      # [2048, 256]
    attn  = softmax(SCALE * query @ key^T)          # row-constant terms cancel
    out   = attn @ key + query

Device-side strategy (v5):
  - The host uploads d1^T and d2^T in bf16 (plus bf16-packed weights and
    f32 biases), so the device does NO input casts and NO input
    transposes, and input HBM traffic halves to ~5.6 MiB/core.  Load
    order (one sync-ring stream): weights/biases, d2T, then d1T in four
    ic-pair chunks so Q projection can start before the tail lands.
  - softmax(q@(k+bk)^T) drops bias terms constant along k, and
    sum(attn)==1 makes attn@(key+bk) == attn@key + bk, so the key value
    matrix carries NO bias; bq biases the scores path and (bq+bk) the
    residual path from the same QT PSUM.  The softmax denominator is a
    memset 1.0 column appended to the fp8 key values.
  - QT for the first q-half accumulates in 4 persistent PSUM banks as
    the d1T chunks arrive; the second half runs as filler between
    scores units.  scoresT [k, q] orientation lets exp(scoresT) feed the
    context matmul as the stationary operand; scores and context run in
    fp8e4m3 DoubleRow.
  - Residual Q reaches [q, d] layout via xbar DMA transposes
    (dma_start_transpose) on the idle mid-kernel DMA engines: no PE, no
    PSUM, no DVE eviction.
  - ctx for the second q-half is split kp0-6 (inside the exp stream,
    evicted to bf16) + kp7 (after the last exp); the post-exp divide
    runs on the then-idle ACT engine (activation scale=reciprocal AP).
  - Output is written in 8 chunks of 256 rows as each completes.
"""

import sys

if "/opt/trn_rl_repo" not in sys.path:
    sys.path.insert(0, "/opt/trn_rl_repo")

from contextlib import ExitStack

import ml_dtypes
import numpy as np

import concourse.bass as bass  # noqa: F401
import concourse.mybir as mybir
import concourse.tile as tile
from concourse import bacc
from concourse.bass_utils import run_bass_kernel_spmd

B, LQ, LK, DIN, D = 8, 2048, 2048, 1024, 256
N_CORES = 8
SCALE = float(1.0 / np.sqrt(1024.0).astype(np.float32))

BF16 = mybir.dt.bfloat16
FP8 = mybir.dt.float8e4
F32 = mybir.dt.float32
AF = mybir.ActivationFunctionType
PM_DR = mybir.MatmulPerfMode.DoubleRow
ADD = mybir.AluOpType.add
MULT = mybir.AluOpType.mult

QB = LQ // 128   # 16 q blocks
KB = LK // 128   # 16 k blocks
IC1 = DIN // 128  # 8
IC2 = D // 128    # 2
KP = KB // 2      # 8 fp8 DoubleRow k-pairs
KP_A = 7          # h1 ctx kp-split: A = kp0-6 inside exp stream, B = kp7 after


def _build():
    nc = bacc.Bacc("TRN2", target_bir_lowering=False, debug=False)
    d1t = nc.dram_tensor("d1t", [DIN, LQ], BF16, kind="ExternalInput").ap()
    d2t = nc.dram_tensor("d2t", [D, LK], BF16, kind="ExternalInput").ap()
    wqk = nc.dram_tensor("wqk", [128, 2560], BF16, kind="ExternalInput").ap()
    bias = nc.dram_tensor("bias", [128, 4], F32, kind="ExternalInput").ap()
    out = nc.dram_tensor("out", [LQ, D], F32, kind="ExternalOutput").ap()

    with tile.TileContext(nc) as tc, ExitStack() as ctx:
        const = ctx.enter_context(tc.tile_pool(name="const", bufs=1))
        big = ctx.enter_context(tc.tile_pool(name="big", bufs=1))
        stage = ctx.enter_context(tc.tile_pool(name="stage", bufs=3))
        small = ctx.enter_context(tc.tile_pool(name="small", bufs=4))
        ps_a = ctx.enter_context(tc.tile_pool(name="ps_a", bufs=4, space="PSUM"))
        ps_sc = ctx.enter_context(tc.tile_pool(name="ps_sc", bufs=2, space="PSUM"))

        # ---------------- constants / small state ---------------------------
        warm_src = const.tile([128, 512], BF16, tag="warm_src")
        nc.gpsimd.memset(warm_src[:], 0.0)
        dummy = const.tile([128, 1], F32, tag="dummy")
        # force the exp ACT table load at kernel start (otherwise it stalls
        # the first real exp by ~1.3us mid-stream)
        nc.scalar.activation(dummy[:], warm_src[:, 0:1], AF.Exp)

        key2 = [
            big.tile([128, 2, D + 1], FP8, tag=f"key2_{kp}", name=f"key2_{kp}")
            for kp in range(KP)
        ]
        for kp in range(KP):
            nc.gpsimd.memset(key2[kp][:, :, D:D + 1], 1.0)

        # ---------------- loads ---------------------------------------------
        wqk_sb = const.tile([128, 2560], BF16, tag="wqk_sb")
        bias_sb = const.tile([128, 4], F32, tag="bias_sb")
        d2T = big.tile([128, IC2, LK], BF16, tag="d2T")
        d1T = [big.tile([128, 2, LQ], BF16, tag=f"d1T{c}", name=f"d1T{c}")
               for c in range(4)]

        nc.sync.dma_start(out=wqk_sb[:], in_=wqk)
        nc.sync.dma_start(out=bias_sb[:], in_=bias)
        nc.sync.dma_start(
            out=d2T[:], in_=d2t.rearrange("(i p) q -> p i q", p=128))
        for c in range(4):
            nc.sync.dma_start(
                out=d1T[c][:],
                in_=d1t[c * 256:(c + 1) * 256, :].rearrange(
                    "(i p) q -> p i q", p=128))

        wqs = [wqk_sb[:, i * D:(i + 1) * D] for i in range(IC1)]
        wks = [wqk_sb[:, 2048 + i * D:2048 + (i + 1) * D] for i in range(IC2)]
        bq_col = bias_sb[:, 0:2]
        bqk_col = bias_sb[:, 2:4]

        # ---------------- PE p-state warmup ---------------------------------
        for w in range(8):
            pw = ps_a.tile([128, 512], F32, tag="ps_a", name=f"warm{w}")
            nc.tensor.matmul(pw[:], lhsT=warm_src[:, :128], rhs=warm_src[:],
                             start=True, stop=True)

        # ---------------- K^T fp8 DoubleRow layout [128, 2, k] --------------
        kt_sb = big.tile([128, 2, LK], FP8, tag="kt_sb")

        def kt_unit(dc, nk):
            ps = ps_a.tile([128, 512], F32, tag="ps_a")
            for ic in range(IC2):
                nc.tensor.matmul(
                    ps[:],
                    lhsT=wks[ic][:, dc * 128:(dc + 1) * 128],
                    rhs=d2T[:, ic, nk * 512:(nk + 1) * 512],
                    start=(ic == 0),
                    stop=(ic == IC2 - 1),
                )
            # ACT is idle pre-exp; keep DVE free for later
            nc.scalar.copy(kt_sb[:, dc, nk * 512:(nk + 1) * 512], ps[:])

        # ---------------- key value pairs (no bias) --------------------------
        def key_pair(kp):
            ps = ps_a.tile([128, 512], F32, tag="ps_a")
            for s in range(2):
                kb = kp * 2 + s
                p = ps[:, s * D:(s + 1) * D]
                for ic in range(IC2):
                    nc.tensor.matmul(
                        p,
                        lhsT=d2T[:, ic, kb * 128:(kb + 1) * 128],
                        rhs=wks[ic],
                        start=(ic == 0),
                        stop=(ic == IC2 - 1),
                    )
            nc.vector.tensor_copy(
                key2[kp][:, :, :D],
                ps[:].rearrange("p (s d) -> p s d", s=2),
            )

        # ---------------- QT projection ------------------------------------
        qt_sb = big.tile([128, 2, LQ], FP8, tag="qt_sb")
        qtbf = big.tile([128, 2, LQ], BF16, tag="qtbf")

        def qt_bias_sc(ps, dc, nq, on_act):
            o = qt_sb[:, dc, nq * 512:(nq + 1) * 512]
            if on_act:
                nc.scalar.activation(o, ps[:], AF.Identity,
                                     bias=bq_col[:, dc:dc + 1])
            else:
                nc.vector.tensor_scalar(o, ps[:], bq_col[:, dc:dc + 1], None, ADD)

        def qt_bias_rs(ps, dc, nq, on_act):
            o = qtbf[:, dc, nq * 512:(nq + 1) * 512]
            if on_act:
                nc.scalar.activation(o, ps[:], AF.Identity,
                                     bias=bqk_col[:, dc:dc + 1])
            else:
                nc.vector.tensor_scalar(o, ps[:], bqk_col[:, dc:dc + 1], None, ADD)

        # -- first q-half: 4 persistent accumulators fed per d1T chunk -------
        qt01_ps = {}

        def qt01_chunk(c):
            for dc in range(2):
                for nq in range(2):
                    if c == 0:
                        qt01_ps[(dc, nq)] = ps_a.tile(
                            [128, 512], F32, tag="ps_a", name=f"qt01_{dc}_{nq}")
                    ps = qt01_ps[(dc, nq)]
                    for i in range(2):
                        nc.tensor.matmul(
                            ps[:],
                            lhsT=wqs[2 * c + i][:, dc * 128:(dc + 1) * 128],
                            rhs=d1T[c][:, i, nq * 512:(nq + 1) * 512],
                            start=(c == 0 and i == 0),
                            stop=(c == 3 and i == 1),
                        )

        # -- second q-half: normal chained units (filler during scores) -----
        def qt23_unit(dc, nq):
            ps = ps_a.tile([128, 512], F32, tag="ps_a")
            for ic in range(IC1):
                c, i = divmod(ic, 2)
                nc.tensor.matmul(
                    ps[:],
                    lhsT=wqs[ic][:, dc * 128:(dc + 1) * 128],
                    rhs=d1T[c][:, i, nq * 512:(nq + 1) * 512],
                    start=(ic == 0),
                    stop=(ic == IC1 - 1),
                )
            qt_bias_sc(ps, dc, nq, on_act=False)
            qt_bias_rs(ps, dc, nq, on_act=False)

        # ---------------- residual Q via xbar DMA transpose ------------------
        # qres3[qg][q_low, j, dc, c] = Q[qg*512 + j*128 + q_low, dc*128 + c]
        qres3 = [big.tile([128, 4, 2, 128], BF16, tag=f"qres{qg}",
                          name=f"qres{qg}")
                 for qg in range(4)]

        def qres_xbar(qg, dc):
            nc.sync.dma_start_transpose(
                out=qres3[qg][:, :, dc, :],
                in_=qtbf[:, dc, qg * 512:(qg + 1) * 512],
            )

        # ---------------- scores + exp --------------------------------------
        expT = [
            [big.tile([128, 2, 1024], FP8, tag=f"expT{kp}_{nh}",
                      name=f"expT{kp}_{nh}")
             for nh in range(2)]
            for kp in range(KP)
        ]

        def scores_unit(km, nh):
            ps = ps_sc.tile([128, 1024], F32, tag="ps_sc")
            for half in range(2):
                nq = nh * 2 + half
                nc.tensor.matmul(
                    ps[:, half * 512:(half + 1) * 512],
                    lhsT=kt_sb[:, :, km * 128:(km + 1) * 128],
                    rhs=qt_sb[:, :, nq * 512:(nq + 1) * 512],
                    perf_mode=PM_DR,
                    start=True,
                    stop=True,
                )
            nc.scalar.activation(
                expT[km // 2][nh][:, km % 2, :], ps[:], AF.Exp, scale=SCALE
            )

        # ---------------- context + residual + out DMA ----------------------
        out_c = [stage.tile([128, 2 * D], F32, tag="outc", name=f"outc{c}")
                 for c in range(QB // 2)]
        ctxA = [big.tile([128, D + 1], BF16, tag=f"ctxA{i}", name=f"ctxA{i}")
                for i in range(8)]

        def ctx_mm(pc, qb, kp, start, stop):
            h, qq = qb // 8, qb % 8
            nc.tensor.matmul(
                pc,
                lhsT=expT[kp][h][:, :, qq * 128:(qq + 1) * 128],
                rhs=key2[kp][:],
                perf_mode=PM_DR,
                start=start,
                stop=stop,
            )

        def ctx_finish(pc, qb, on_act=False):
            rc = small.tile([128, 1], F32, tag="recip")
            nc.vector.reciprocal(rc[:], pc[:, D:D + 1])
            c = qb // 2
            osl = out_c[c][:, (qb % 2) * D:(qb % 2 + 1) * D]
            if on_act:
                nc.scalar.activation(osl, pc[:, :D], AF.Identity, scale=rc[:])
            else:
                nc.vector.tensor_scalar(osl, pc[:, :D], rc[:], None, MULT)
            qg, j = qb // 4, qb % 4
            nc.gpsimd.tensor_add(
                osl.rearrange("p (a b) -> p a b", a=2),
                osl.rearrange("p (a b) -> p a b", a=2),
                qres3[qg][:, j, :, :],
            )
            if qb % 2 == 1:
                nc.sync.dma_start(
                    out=out[c * 256:(c + 1) * 256, :].rearrange(
                        "(t p) d -> p t d", p=128),
                    in_=out_c[c][:].rearrange("p (t d) -> p t d", d=D),
                )

        def ctx_unit_h0(qb):
            pc_full = ps_a.tile([128, 512], F32, tag="ps_a")
            pc = pc_full[:, :D + 1]
            for kp in range(KP):
                ctx_mm(pc, qb, kp, kp == 0, kp == KP - 1)
            ctx_finish(pc, qb)

        def ctx_h1_A(qb):
            pc_full = ps_a.tile([128, 512], F32, tag="ps_a")
            pc = pc_full[:, :D + 1]
            for kp in range(KP_A):
                ctx_mm(pc, qb, kp, kp == 0, kp == KP_A - 1)
            nc.vector.tensor_copy(ctxA[qb - 8][:], pc)

        def ctx_h1_B(qb):
            pc_full = ps_a.tile([128, 512], F32, tag="ps_a")
            pc = pc_full[:, :D + 1]
            for kp in range(KP_A, KP):
                ctx_mm(pc, qb, kp, kp == KP_A, kp == KP - 1)
            nc.vector.tensor_tensor(pc, pc, ctxA[qb - 8][:], ADD)
            ctx_finish(pc, qb, on_act=True)

        # ================= emission schedule ================================
        def units(fn, idxs):
            return [lambda i=i: fn(*i) if isinstance(i, tuple) else fn(i)
                    for i in idxs]

        def interleave(a, b, ratio):
            a = list(a)
            b = list(b)
            ia = ib = 0
            credit = 0.0
            while ia < len(a) or ib < len(b):
                if ia < len(a):
                    a[ia]()
                    ia += 1
                credit += ratio
                while credit >= 1.0 and ib < len(b):
                    b[ib]()
                    ib += 1
                    credit -= 1.0
            while ib < len(b):
                b[ib]()
                ib += 1

        # --- phase 1: KT + keypair (d2T), then QT01 chunks as d1T lands ---
        for nk in range(4):
            for dc in range(2):
                kt_unit(dc, nk)
        for kp in range(KP):
            key_pair(kp)
        for c in range(4):
            qt01_chunk(c)
        # scores-path evicts first (they gate exp #0), on ACT; residual-path
        # evicts on DVE in parallel
        for dc in range(2):
            for nq in range(2):
                qt_bias_sc(qt01_ps[(dc, nq)], dc, nq, on_act=True)
        for dc in range(2):
            for nq in range(2):
                qt_bias_rs(qt01_ps[(dc, nq)], dc, nq, on_act=False)
        for qg in range(2):
            for dc in range(2):
                qres_xbar(qg, dc)

        # --- phase 2: scores-h0 interleaved with QT23 + late qres ---
        def qres_late(qg):
            for dc in range(2):
                qres_xbar(qg, dc)

        filler = (
            units(qt23_unit, [(dc, nq) for nq in (2, 3) for dc in range(2)])
            + [lambda: qres_late(2), lambda: qres_late(3)]
        )
        scores_h0 = units(scores_unit, [(km, 0) for km in range(KB)])
        interleave(scores_h0, filler, len(filler) / len(scores_h0))

        # --- phase 3: scores-h1 with ctx-h0 and ctx-h1-A ---
        # ctx_h1_A reads exps km0-13 of h1, so A units may only be emitted
        # after scores-h1 km13 (Tile orders by emission-time dependencies)
        sc_h1_a = units(scores_unit, [(km, 1) for km in range(2 * KP_A)])
        sc_h1_b = units(scores_unit, [(km, 1) for km in range(2 * KP_A, KB)])
        ctx0 = units(ctx_unit_h0, list(range(0, 8)))
        ctxa = units(ctx_h1_A, list(range(8, 16)))
        interleave(sc_h1_a, ctx0, len(ctx0) / len(sc_h1_a))
        interleave(sc_h1_b, ctxa, len(ctxa) / len(sc_h1_b))

        # --- phase 4: ctx-h1-B tail ---
        for qb in range(8, 16):
            ctx_h1_B(qb)

    nc.compile()
    return nc


_NC = None
_last_in_maps = None


def make_host_inputs(data1_b, data2_b, Wq, bq, Wk, bk):
    """Pack one batch element's inputs into the device layout (bf16 + f32)."""
    bf = ml_dtypes.bfloat16
    d1t = np.ascontiguousarray(np.asarray(data1_b, np.float32).astype(bf).T)
    d2t = np.ascontiguousarray(np.asarray(data2_b, np.float32).astype(bf).T)
    Wq = np.asarray(Wq, dtype=np.float32)
    Wk = np.asarray(Wk, dtype=np.float32)
    bq = np.asarray(bq, dtype=np.float32)
    bk = np.asarray(bk, dtype=np.float32)
    wqk = np.empty((128, 2560), bf)
    for c in range(IC1):
        wqk[:, c * 256:(c + 1) * 256] = Wq[c * 128:(c + 1) * 128, :].astype(bf)
    for c in range(IC2):
        wqk[:, 2048 + c * 256:2048 + (c + 1) * 256] = (
            Wk[c * 128:(c + 1) * 128, :].astype(bf))
    bias = np.empty((128, 4), np.float32)
    bqk = bq + bk
    for c in range(2):
        bias[:, c] = bq[c * 128:(c + 1) * 128]
        bias[:, 2 + c] = bqk[c * 128:(c + 1) * 128]
    return {"d1t": d1t, "d2t": d2t, "wqk": wqk, "bias": bias}


def _get_nc():
    global _NC
    if _NC is None:
        _NC = _build()
    return _NC


def kernel(data1, data2, Wq, bq, Wk, bk):
    global _last_in_maps
    data1 = np.asarray(data1, dtype=np.float32)
    data2 = np.asarray(data2, dtype=np.float32)

    nc = _get_nc()
    shared = None
    in_maps = []
    for b in range(B):
        m = make_host_inputs(data1[b], data2[b], Wq, bq, Wk, bk)
        if shared is None:
            shared = {"wqk": m["wqk"], "bias": m["bias"]}
        m["wqk"] = shared["wqk"]
        m["bias"] = shared["bias"]
        in_maps.append(m)
    _last_in_maps = in_maps
    res = run_bass_kernel_spmd(nc, in_maps, core_ids=list(range(N_CORES)))
    return np.stack([res.results[i]["out"] for i in range(B)], axis=0)


# revision 5
# speedup vs baseline: 1.3067x; 1.0278x over previous
"""CoAttention kernel for Trainium2, data-parallel over batch across 8 NeuronCores.

Per core (one batch element b):
    query = data1[b] @ Wq + bq                      # [2048, 256]
    key   = data2[b] @ Wk + bk                      # [2048, 256]
    attn  = softmax(SCALE * query @ key^T)          # row-constant terms cancel
    out   = attn @ key + query

Device-side strategy (v6):
  - The host uploads d1^T and d2^T in bf16 (plus bf16 weights and f32
    biases), so the device does NO input casts and NO input transposes,
    and input HBM traffic halves to ~5.6 MiB/core.  Load order (one
    sync-ring stream): wk, d2T, wq, biases, then d1T in four ic-pair
    chunks; compute is sequenced so each load is consumed as it lands.
  - softmax(q@(k+bk)^T) drops bias terms constant along k, and
    sum(attn)==1 makes attn@(key+bk) == attn@key + bk, so the key value
    matrix carries NO bias; bq biases the scores path and (bq+bk) the
    residual path from the same QT PSUM.  The softmax denominator is a
    memset 1.0 column appended to the fp8 key values.
  - QT for the first q-half accumulates in the two scores PSUM banks as
    the d1T chunks arrive (evicted as one [128,1024] pass per dc); the
    second half runs as filler between scores units.  key values come
    from fp8 PE transposes of kt (no second projection matmul).
  - Residual Q reaches [q, d] layout via xbar DMA transposes on the
    idle mid-kernel DMA engines: no PE, no PSUM, no DVE eviction.
  - scoresT [k, q] orientation lets exp(scoresT) feed the context
    matmul as the stationary operand; scores and context run in fp8e4m3
    DoubleRow.  ctx for the second q-half is split kp0-6 (inside the
    exp stream, evicted to bf16) + kp7 (after the last exp); post-exp
    divides run on the then-idle ACT engine (activation scale=recip AP).
  - Output is written in 8 chunks of 256 rows as each completes.
"""

import sys

if "/opt/trn_rl_repo" not in sys.path:
    sys.path.insert(0, "/opt/trn_rl_repo")

from contextlib import ExitStack

import ml_dtypes
import numpy as np

import concourse.bass as bass  # noqa: F401
import concourse.mybir as mybir
import concourse.tile as tile
from concourse import bacc
from concourse.bass_utils import run_bass_kernel_spmd
from concourse.masks import make_identity

B, LQ, LK, DIN, D = 8, 2048, 2048, 1024, 256
N_CORES = 8
SCALE = float(1.0 / np.sqrt(1024.0).astype(np.float32))

BF16 = mybir.dt.bfloat16
FP8 = mybir.dt.float8e4
F32 = mybir.dt.float32
AF = mybir.ActivationFunctionType
PM_DR = mybir.MatmulPerfMode.DoubleRow
ADD = mybir.AluOpType.add
MULT = mybir.AluOpType.mult

QB = LQ // 128   # 16 q blocks
KB = LK // 128   # 16 k blocks
IC1 = DIN // 128  # 8
IC2 = D // 128    # 2
KP = KB // 2      # 8 fp8 DoubleRow k-pairs
KP_A = 7          # h1 ctx kp-split: A = kp0-6 inside exp stream, B = kp7 after


def _build():
    nc = bacc.Bacc("TRN2", target_bir_lowering=False, debug=False)
    d1t = nc.dram_tensor("d1t", [DIN, LQ], BF16, kind="ExternalInput").ap()
    d2t = nc.dram_tensor("d2t", [D, LK], BF16, kind="ExternalInput").ap()
    wq_d = nc.dram_tensor("wq", [128, 2048], BF16, kind="ExternalInput").ap()
    wk_d = nc.dram_tensor("wk", [128, 512], BF16, kind="ExternalInput").ap()
    bias = nc.dram_tensor("bias", [128, 4], F32, kind="ExternalInput").ap()
    out = nc.dram_tensor("out", [LQ, D], F32, kind="ExternalOutput").ap()

    with tile.TileContext(nc) as tc, ExitStack() as ctx:
        const = ctx.enter_context(tc.tile_pool(name="const", bufs=1))
        big = ctx.enter_context(tc.tile_pool(name="big", bufs=1))
        stage = ctx.enter_context(tc.tile_pool(name="stage", bufs=3))
        small = ctx.enter_context(tc.tile_pool(name="small", bufs=4))
        ps_a = ctx.enter_context(tc.tile_pool(name="ps_a", bufs=3, space="PSUM"))
        ps_t = ctx.enter_context(tc.tile_pool(name="ps_t", bufs=1, space="PSUM"))
        ps_sc = ctx.enter_context(tc.tile_pool(name="ps_sc", bufs=2, space="PSUM"))

        # ---------------- constants / small state ---------------------------
        warm_src = const.tile([128, 512], BF16, tag="warm_src")
        nc.gpsimd.memset(warm_src[:], 0.0)
        ident_f8 = const.tile([128, 128], FP8, tag="ident_f8")
        make_identity(nc, ident_f8[:])
        dummy = const.tile([128, 1], F32, tag="dummy")
        # force the exp ACT table load at kernel start (otherwise it stalls
        # the first real exp by ~1.3us mid-stream)
        nc.scalar.activation(dummy[:], warm_src[:, 0:1], AF.Exp)

        key2 = [
            big.tile([128, 2, D + 1], FP8, tag=f"key2_{kp}", name=f"key2_{kp}")
            for kp in range(KP)
        ]
        for kp in range(KP):
            nc.gpsimd.memset(key2[kp][:, :, D:D + 1], 1.0)

        # ---------------- loads ---------------------------------------------
        wq_sb = const.tile([128, 2048], BF16, tag="wq_sb")
        wk_sb = const.tile([128, 512], BF16, tag="wk_sb")
        bias_sb = const.tile([128, 4], F32, tag="bias_sb")
        d2T = big.tile([128, IC2, LK], BF16, tag="d2T")
        d1T = [big.tile([128, 2, LQ], BF16, tag=f"d1T{c}", name=f"d1T{c}")
               for c in range(4)]

        nc.sync.dma_start(out=wk_sb[:], in_=wk_d)
        nc.sync.dma_start(
            out=d2T[:], in_=d2t.rearrange("(i p) q -> p i q", p=128))
        nc.sync.dma_start(out=wq_sb[:], in_=wq_d)
        nc.sync.dma_start(out=bias_sb[:], in_=bias)
        for c in range(4):
            nc.sync.dma_start(
                out=d1T[c][:],
                in_=d1t[c * 256:(c + 1) * 256, :].rearrange(
                    "(i p) q -> p i q", p=128))

        wqs = [wq_sb[:, i * D:(i + 1) * D] for i in range(IC1)]
        wks = [wk_sb[:, i * D:(i + 1) * D] for i in range(IC2)]
        bq_col = bias_sb[:, 0:2]
        bqk_col = bias_sb[:, 2:4]

        # ---------------- PE p-state warmup ---------------------------------
        for w in range(14):
            pw = ps_a.tile([128, 512], F32, tag="ps_a", name=f"warm{w}")
            nc.tensor.matmul(pw[:], lhsT=warm_src[:, :128], rhs=warm_src[:],
                             start=True, stop=True)

        # ---------------- K^T fp8 DoubleRow layout [128, 2, k] --------------
        kt_sb = big.tile([128, 2, LK], FP8, tag="kt_sb")

        def kt_unit(dc, nk):
            ps = ps_a.tile([128, 512], F32, tag="ps_a")
            for ic in range(IC2):
                nc.tensor.matmul(
                    ps[:],
                    lhsT=wks[ic][:, dc * 128:(dc + 1) * 128],
                    rhs=d2T[:, ic, nk * 512:(nk + 1) * 512],
                    start=(ic == 0),
                    stop=(ic == IC2 - 1),
                )
            # ACT is idle pre-exp; keep DVE free for later
            nc.scalar.copy(kt_sb[:, dc, nk * 512:(nk + 1) * 512], ps[:])

        # ---------------- key values via fp8 PE transpose of kt --------------
        def key_tr(kp):
            # fp8 PE transpose requires output element step 2 in PSUM
            ps = ps_t.tile([128, 512, 2], FP8, tag="ps_t")
            for s in range(2):
                for dc in range(IC2):
                    nc.tensor.transpose(
                        ps[:, s * 256 + dc * 128: s * 256 + (dc + 1) * 128, 0],
                        kt_sb[:, dc, (2 * kp + s) * 128:(2 * kp + s + 1) * 128],
                        ident_f8[:],
                    )
            nc.vector.tensor_copy(
                key2[kp][:, :, :D],
                ps[:, :, 0].rearrange("p (s d) -> p s d", s=2),
            )

        # ---------------- QT projection ------------------------------------
        qt_sb = big.tile([128, 2, LQ], FP8, tag="qt_sb")
        qtbf = big.tile([128, 2, LQ], BF16, tag="qtbf")

        # -- first q-half: accumulate in the two scores PSUM banks as the
        #    d1T chunks arrive; evict as one [128,1024] pass per dc.
        qt01_ps = {}

        def qt01_chunk(c):
            for dc in range(2):
                for nq in range(2):
                    if c == 0 and nq == 0:
                        qt01_ps[dc] = ps_sc.tile(
                            [128, 1024], F32, tag="ps_sc", name=f"qt01_{dc}")
                    ps = qt01_ps[dc]
                    for i in range(2):
                        nc.tensor.matmul(
                            ps[:, nq * 512:(nq + 1) * 512],
                            lhsT=wqs[2 * c + i][:, dc * 128:(dc + 1) * 128],
                            rhs=d1T[c][:, i, nq * 512:(nq + 1) * 512],
                            start=(c == 0 and i == 0),
                            stop=(c == 3 and i == 1),
                        )

        def qt01_evict(dc):
            # scores-path (gates exp #0) on ACT; residual-path on DVE
            nc.scalar.activation(qt_sb[:, dc, 0:1024], qt01_ps[dc][:],
                                 AF.Identity, bias=bq_col[:, dc:dc + 1])
            nc.vector.tensor_scalar(qtbf[:, dc, 0:1024], qt01_ps[dc][:],
                                    bqk_col[:, dc:dc + 1], None, ADD)

        # -- second q-half: normal chained units (filler during scores) -----
        def qt23_unit(dc, nq):
            ps = ps_a.tile([128, 512], F32, tag="ps_a")
            for ic in range(IC1):
                c, i = divmod(ic, 2)
                nc.tensor.matmul(
                    ps[:],
                    lhsT=wqs[ic][:, dc * 128:(dc + 1) * 128],
                    rhs=d1T[c][:, i, nq * 512:(nq + 1) * 512],
                    start=(ic == 0),
                    stop=(ic == IC1 - 1),
                )
            nc.vector.tensor_scalar(qt_sb[:, dc, nq * 512:(nq + 1) * 512],
                                    ps[:], bq_col[:, dc:dc + 1], None, ADD)
            nc.vector.tensor_scalar(qtbf[:, dc, nq * 512:(nq + 1) * 512],
                                    ps[:], bqk_col[:, dc:dc + 1], None, ADD)

        # ---------------- residual Q via xbar DMA transpose ------------------
        # qres3[qg][q_low, j, dc, c] = Q[qg*512 + j*128 + q_low, dc*128 + c]
        qres3 = [big.tile([128, 4, 2, 128], BF16, tag=f"qres{qg}",
                          name=f"qres{qg}")
                 for qg in range(4)]

        def qres_xbar(qg, dc):
            nc.sync.dma_start_transpose(
                out=qres3[qg][:, :, dc, :],
                in_=qtbf[:, dc, qg * 512:(qg + 1) * 512],
            )

        # ---------------- scores + exp --------------------------------------
        expT = [
            [big.tile([128, 2, 1024], FP8, tag=f"expT{kp}_{nh}",
                      name=f"expT{kp}_{nh}")
             for nh in range(2)]
            for kp in range(KP)
        ]

        def scores_unit(km, nh):
            ps = ps_sc.tile([128, 1024], F32, tag="ps_sc")
            for half in range(2):
                nq = nh * 2 + half
                nc.tensor.matmul(
                    ps[:, half * 512:(half + 1) * 512],
                    lhsT=kt_sb[:, :, km * 128:(km + 1) * 128],
                    rhs=qt_sb[:, :, nq * 512:(nq + 1) * 512],
                    perf_mode=PM_DR,
                    start=True,
                    stop=True,
                )
            nc.scalar.activation(
                expT[km // 2][nh][:, km % 2, :], ps[:], AF.Exp, scale=SCALE
            )

        # ---------------- context + residual + out DMA ----------------------
        out_c = [stage.tile([128, 2 * D], F32, tag="outc", name=f"outc{c}")
                 for c in range(QB // 2)]
        ctxA = [big.tile([128, D + 1], BF16, tag=f"ctxA{i}", name=f"ctxA{i}")
                for i in range(8)]

        def ctx_mm(pc, qb, kp, start, stop):
            h, qq = qb // 8, qb % 8
            nc.tensor.matmul(
                pc,
                lhsT=expT[kp][h][:, :, qq * 128:(qq + 1) * 128],
                rhs=key2[kp][:],
                perf_mode=PM_DR,
                start=start,
                stop=stop,
            )

        def ctx_finish(pc, qb, div_act=False, add_dve=False):
            rc = small.tile([128, 1], F32, tag="recip")
            nc.vector.reciprocal(rc[:], pc[:, D:D + 1])
            c = qb // 2
            osl = out_c[c][:, (qb % 2) * D:(qb % 2 + 1) * D]
            if div_act:
                nc.scalar.activation(osl, pc[:, :D], AF.Identity, scale=rc[:])
            else:
                nc.vector.tensor_scalar(osl, pc[:, :D], rc[:], None, MULT)
            qg, j = qb // 4, qb % 4
            o2 = osl.rearrange("p (a b) -> p a b", a=2)
            if add_dve:
                nc.vector.tensor_tensor(o2, o2, qres3[qg][:, j, :, :], ADD)
            else:
                nc.gpsimd.tensor_add(o2, o2, qres3[qg][:, j, :, :])
            if qb % 2 == 1:
                nc.sync.dma_start(
                    out=out[c * 256:(c + 1) * 256, :].rearrange(
                        "(t p) d -> p t d", p=128),
                    in_=out_c[c][:].rearrange("p (t d) -> p t d", d=D),
                )

        def ctx_unit_h0(qb):
            pc_full = ps_a.tile([128, 512], F32, tag="ps_a")
            pc = pc_full[:, :D + 1]
            for kp in range(KP):
                ctx_mm(pc, qb, kp, kp == 0, kp == KP - 1)
            ctx_finish(pc, qb)

        def ctx_h1_A(qb):
            pc_full = ps_a.tile([128, 512], F32, tag="ps_a")
            pc = pc_full[:, :D + 1]
            for kp in range(KP_A):
                ctx_mm(pc, qb, kp, kp == 0, kp == KP_A - 1)
            nc.vector.tensor_copy(ctxA[qb - 8][:], pc)

        def ctx_h1_B(qb):
            pc_full = ps_a.tile([128, 512], F32, tag="ps_a")
            pc = pc_full[:, :D + 1]
            for kp in range(KP_A, KP):
                ctx_mm(pc, qb, kp, kp == KP_A, kp == KP - 1)
            nc.vector.tensor_tensor(pc, pc, ctxA[qb - 8][:], ADD)
            ctx_finish(pc, qb, div_act=True, add_dve=(qb % 4 >= 2))

        # ================= emission schedule ================================
        def units(fn, idxs):
            return [lambda i=i: fn(*i) if isinstance(i, tuple) else fn(i)
                    for i in idxs]

        def interleave(a, b, ratio):
            a = list(a)
            b = list(b)
            ia = ib = 0
            credit = 0.0
            while ia < len(a) or ib < len(b):
                if ia < len(a):
                    a[ia]()
                    ia += 1
                credit += ratio
                while credit >= 1.0 and ib < len(b):
                    b[ib]()
                    ib += 1
                    credit -= 1.0
            while ib < len(b):
                b[ib]()
                ib += 1

        # --- phase 1: KT as d2T lands, then QT01 chunks as d1T lands ---
        for nk in range(4):
            for dc in range(2):
                kt_unit(dc, nk)
        for c in range(4):
            qt01_chunk(c)
        for dc in range(2):
            qt01_evict(dc)
        for qg in range(2):
            for dc in range(2):
                qres_xbar(qg, dc)

        # --- phase 2: scores-h0 interleaved with QT23 + key transposes ---
        def qres_late(qg):
            for dc in range(2):
                qres_xbar(qg, dc)

        filler = (
            units(qt23_unit, [(dc, nq) for nq in (2, 3) for dc in range(2)])
            + [lambda: qres_late(2), lambda: qres_late(3)]
            + units(key_tr, list(range(KP)))
        )
        scores_h0 = units(scores_unit, [(km, 0) for km in range(KB)])
        interleave(scores_h0, filler, len(filler) / len(scores_h0))

        # --- phase 3: scores-h1 with ctx-h0 and ctx-h1-A ---
        # ctx_h1_A reads exps km0-13 of h1, so A units may only be emitted
        # after scores-h1 km13 (Tile orders by emission-time dependencies)
        sc_h1_a = units(scores_unit, [(km, 1) for km in range(2 * KP_A)])
        sc_h1_b = units(scores_unit, [(km, 1) for km in range(2 * KP_A, KB)])
        ctx0 = units(ctx_unit_h0, list(range(0, 8)))
        ctxa = units(ctx_h1_A, list(range(8, 16)))
        interleave(sc_h1_a, ctx0, len(ctx0) / len(sc_h1_a))
        interleave(sc_h1_b, ctxa, len(ctxa) / len(sc_h1_b))

        # --- phase 4: ctx-h1-B tail ---
        for qb in range(8, 16):
            ctx_h1_B(qb)

    nc.compile()
    return nc


_NC = None
_last_in_maps = None


def make_host_inputs(data1_b, data2_b, Wq, bq, Wk, bk):
    """Pack one batch element's inputs into the device layout (bf16 + f32)."""
    bf = ml_dtypes.bfloat16
    d1t = np.ascontiguousarray(np.asarray(data1_b, np.float32).astype(bf).T)
    d2t = np.ascontiguousarray(np.asarray(data2_b, np.float32).astype(bf).T)
    Wq = np.asarray(Wq, dtype=np.float32)
    Wk = np.asarray(Wk, dtype=np.float32)
    bq = np.asarray(bq, dtype=np.float32)
    bk = np.asarray(bk, dtype=np.float32)
    wq = np.empty((128, 2048), bf)
    for c in range(IC1):
        wq[:, c * 256:(c + 1) * 256] = Wq[c * 128:(c + 1) * 128, :].astype(bf)
    wk = np.empty((128, 512), bf)
    for c in range(IC2):
        wk[:, c * 256:(c + 1) * 256] = Wk[c * 128:(c + 1) * 128, :].astype(bf)
    bias = np.empty((128, 4), np.float32)
    bqk = bq + bk
    for c in range(2):
        bias[:, c] = bq[c * 128:(c + 1) * 128]
        bias[:, 2 + c] = bqk[c * 128:(c + 1) * 128]
    return {"d1t": d1t, "d2t": d2t, "wq": wq, "wk": wk, "bias": bias}


def _get_nc():
    global _NC
    if _NC is None:
        _NC = _build()
    return _NC


def kernel(data1, data2, Wq, bq, Wk, bk):
    global _last_in_maps
    data1 = np.asarray(data1, dtype=np.float32)
    data2 = np.asarray(data2, dtype=np.float32)

    nc = _get_nc()
    shared = None
    in_maps = []
    for b in range(B):
        m = make_host_inputs(data1[b], data2[b], Wq, bq, Wk, bk)
        if shared is None:
            shared = {k: m[k] for k in ("wq", "wk", "bias")}
        m.update(shared)
        in_maps.append(m)
    _last_in_maps = in_maps
    res = run_bass_kernel_spmd(nc, in_maps, core_ids=list(range(N_CORES)))
    return np.stack([res.results[i]["out"] for i in range(B)], axis=0)


# revision 7
# speedup vs baseline: 1.3268x; 1.0154x over previous
"""CoAttention kernel for Trainium2, data-parallel over batch across 8 NeuronCores.

Per core (one batch element b):
    query = data1[b] @ Wq + bq                      # [2048, 256]
    key   = data2[b] @ Wk + bk                      # [2048, 256]
    attn  = softmax(SCALE * query @ key^T)          # row-constant terms cancel
    out   = attn @ key + query

Device-side strategy (v7):
  - The host uploads d1^T and d2^T in bf16, i-interleaved p-major so
    every DMA descriptor is one 8KB contiguous run per partition, and
    d1^T is split into four q-chunks so each QT projection unit
    completes as its chunk lands (the weight rows are packed in the
    matching i-permutation; contraction order is free).  No device
    casts, no input transposes, ~5.6 MiB/core input traffic.
  - softmax(q@(k+bk)^T) drops bias terms constant along k, and
    sum(attn)==1 makes attn@(key+bk) == attn@key + bk, so the key value
    matrix carries NO bias; bq biases the scores path and (bq+bk) the
    residual path from the same QT PSUM.  The softmax denominator is a
    memset 1.0 column appended to the fp8 key values.
  - Scores-path QT evicts split ACT/DVE so exp #0 fires ~1us after the
    second d1 chunk lands.  key values come from fp8 PE transposes of
    kt (stride-2 PSUM), not a second projection matmul.
  - Residual Q reaches [q, d] layout via xbar DMA transposes on the
    idle mid-kernel DMA engines: no PE, no PSUM, no DVE eviction.
  - scoresT [k, q] orientation lets exp(scoresT) feed the context
    matmul as the stationary operand; scores and context run in fp8e4m3
    DoubleRow.  ctx for the second q-half is split kp0-6 (inside the
    exp stream, evicted to bf16) + kp7 (after the last exp); post-exp
    divides run on the then-idle ACT engine (activation scale=recip AP).
  - Output is written in 8 chunks of 256 rows as each completes.
"""

import sys

if "/opt/trn_rl_repo" not in sys.path:
    sys.path.insert(0, "/opt/trn_rl_repo")

from contextlib import ExitStack

import ml_dtypes
import numpy as np

import concourse.bass as bass  # noqa: F401
import concourse.mybir as mybir
import concourse.tile as tile
from concourse import bacc
from concourse.bass_utils import run_bass_kernel_spmd
from concourse.masks import make_identity

B, LQ, LK, DIN, D = 8, 2048, 2048, 1024, 256
N_CORES = 8
SCALE = float(1.0 / np.sqrt(1024.0).astype(np.float32))

BF16 = mybir.dt.bfloat16
FP8 = mybir.dt.float8e4
F32 = mybir.dt.float32
AF = mybir.ActivationFunctionType
PM_DR = mybir.MatmulPerfMode.DoubleRow
ADD = mybir.AluOpType.add
MULT = mybir.AluOpType.mult

QB = 16           # q blocks of 128
KB = 16           # k blocks of 128
J1 = 8            # d1 i-interleave factor (1024 = 128 * 8)
J2 = 2            # d2 i-interleave factor (256 = 128 * 2)
KP = KB // 2      # 8 fp8 DoubleRow k-pairs
KP_A = 7          # h1 ctx kp-split: A = kp0-6 inside exp stream, B = kp7 after


def _build():
    nc = bacc.Bacc("TRN2", target_bir_lowering=False, debug=False)
    d1t = nc.dram_tensor("d1t", [4, 128, J1, 512], BF16, kind="ExternalInput").ap()
    d2t = nc.dram_tensor("d2t", [128, J2, LK], BF16, kind="ExternalInput").ap()
    wq_d = nc.dram_tensor("wq", [128, 2048], BF16, kind="ExternalInput").ap()
    wk_d = nc.dram_tensor("wk", [128, 512], BF16, kind="ExternalInput").ap()
    bias = nc.dram_tensor("bias", [128, 4], F32, kind="ExternalInput").ap()
    out = nc.dram_tensor("out", [LQ, D], F32, kind="ExternalOutput").ap()

    with tile.TileContext(nc) as tc, ExitStack() as ctx:
        const = ctx.enter_context(tc.tile_pool(name="const", bufs=1))
        big = ctx.enter_context(tc.tile_pool(name="big", bufs=1))
        stage = ctx.enter_context(tc.tile_pool(name="stage", bufs=3))
        small = ctx.enter_context(tc.tile_pool(name="small", bufs=4))
        ps_a = ctx.enter_context(tc.tile_pool(name="ps_a", bufs=3, space="PSUM"))
        ps_t = ctx.enter_context(tc.tile_pool(name="ps_t", bufs=1, space="PSUM"))
        ps_sc = ctx.enter_context(tc.tile_pool(name="ps_sc", bufs=2, space="PSUM"))

        # ---------------- constants / small state ---------------------------
        warm_src = const.tile([128, 512], BF16, tag="warm_src")
        nc.gpsimd.memset(warm_src[:], 0.0)
        ident_f8 = const.tile([128, 128], FP8, tag="ident_f8")
        make_identity(nc, ident_f8[:])
        dummy = const.tile([128, 1], F32, tag="dummy")
        # force the exp ACT table load at kernel start (otherwise it stalls
        # the first real exp by ~1.3us mid-stream)
        nc.scalar.activation(dummy[:], warm_src[:, 0:1], AF.Exp)

        key2 = [
            big.tile([128, 2, D + 1], FP8, tag=f"key2_{kp}", name=f"key2_{kp}")
            for kp in range(KP)
        ]
        for kp in range(KP):
            nc.gpsimd.memset(key2[kp][:, :, D:D + 1], 1.0)

        # ---------------- loads ---------------------------------------------
        wq_sb = const.tile([128, 2048], BF16, tag="wq_sb")
        wk_sb = const.tile([128, 512], BF16, tag="wk_sb")
        bias_sb = const.tile([128, 4], F32, tag="bias_sb")
        d2T = big.tile([128, J2, LK], BF16, tag="d2T")
        d1T = [big.tile([128, J1, 512], BF16, tag=f"d1T{n}", name=f"d1T{n}")
               for n in range(4)]

        nc.sync.dma_start(out=wk_sb[:], in_=wk_d)
        nc.sync.dma_start(out=d2T[:], in_=d2t)
        nc.sync.dma_start(out=wq_sb[:], in_=wq_d)
        nc.sync.dma_start(out=bias_sb[:], in_=bias)
        for n in range(4):
            nc.sync.dma_start(out=d1T[n][:], in_=d1t[n])

        # weight slices in the same i-permutation as the activations
        wqs = [wq_sb[:, j * D:(j + 1) * D] for j in range(J1)]
        wks = [wk_sb[:, j * D:(j + 1) * D] for j in range(J2)]
        bq_col = bias_sb[:, 0:2]
        bqk_col = bias_sb[:, 2:4]

        # ---------------- PE p-state warmup ---------------------------------
        for w in range(6):
            pw = ps_a.tile([128, 512], F32, tag="ps_a", name=f"warm{w}")
            nc.tensor.matmul(pw[:], lhsT=warm_src[:, :128], rhs=warm_src[:],
                             start=True, stop=True)

        # ---------------- K^T fp8 DoubleRow layout [128, 2, k] --------------
        kt_sb = big.tile([128, 2, LK], FP8, tag="kt_sb")

        def kt_unit(dc, nk):
            ps = ps_a.tile([128, 512], F32, tag="ps_a")
            for j in range(J2):
                nc.tensor.matmul(
                    ps[:],
                    lhsT=wks[j][:, dc * 128:(dc + 1) * 128],
                    rhs=d2T[:, j, nk * 512:(nk + 1) * 512],
                    start=(j == 0),
                    stop=(j == J2 - 1),
                )
            # ACT is idle pre-exp; keep DVE free for later
            nc.scalar.copy(kt_sb[:, dc, nk * 512:(nk + 1) * 512], ps[:])

        # ---------------- key values via fp8 PE transpose of kt --------------
        def key_tr(kp):
            # fp8 PE transpose requires output element step 2 in PSUM
            ps = ps_t.tile([128, 512, 2], FP8, tag="ps_t")
            for s in range(2):
                for dc in range(2):
                    nc.tensor.transpose(
                        ps[:, s * 256 + dc * 128: s * 256 + (dc + 1) * 128, 0],
                        kt_sb[:, dc, (2 * kp + s) * 128:(2 * kp + s + 1) * 128],
                        ident_f8[:],
                    )
            nc.vector.tensor_copy(
                key2[kp][:, :, :D],
                ps[:, :, 0].rearrange("p (s d) -> p s d", s=2),
            )

        # ---------------- QT projection ------------------------------------
        qt_sb = big.tile([128, 2, LQ], FP8, tag="qt_sb")
        qtbf = big.tile([128, 2, LQ], BF16, tag="qtbf")

        def qt_bias_sc(ps, dc, nq, on_act):
            o = qt_sb[:, dc, nq * 512:(nq + 1) * 512]
            if on_act:
                nc.scalar.activation(o, ps[:], AF.Identity,
                                     bias=bq_col[:, dc:dc + 1])
            else:
                nc.vector.tensor_scalar(o, ps[:], bq_col[:, dc:dc + 1], None, ADD)

        def qt_bias_rs(ps, dc, nq, on_act):
            o = qtbf[:, dc, nq * 512:(nq + 1) * 512]
            if on_act:
                nc.scalar.activation(o, ps[:], AF.Identity,
                                     bias=bqk_col[:, dc:dc + 1])
            else:
                nc.vector.tensor_scalar(o, ps[:], bqk_col[:, dc:dc + 1], None, ADD)

        def qt_unit(dc, nq, sc_act, rs_act):
            ps = ps_a.tile([128, 512], F32, tag="ps_a")
            for j in range(J1):
                nc.tensor.matmul(
                    ps[:],
                    lhsT=wqs[j][:, dc * 128:(dc + 1) * 128],
                    rhs=d1T[nq][:, j, :],
                    start=(j == 0),
                    stop=(j == J1 - 1),
                )
            qt_bias_sc(ps, dc, nq, sc_act)
            qt_bias_rs(ps, dc, nq, rs_act)

        # ---------------- residual Q via xbar DMA transpose ------------------
        # qres3[qg][q_low, j, dc, c] = Q[qg*512 + j*128 + q_low, dc*128 + c]
        qres3 = [big.tile([128, 4, 2, 128], BF16, tag=f"qres{qg}",
                          name=f"qres{qg}")
                 for qg in range(4)]

        def qres_xbar(qg, dc):
            nc.sync.dma_start_transpose(
                out=qres3[qg][:, :, dc, :],
                in_=qtbf[:, dc, qg * 512:(qg + 1) * 512],
            )

        # ---------------- scores + exp --------------------------------------
        expT = [
            [big.tile([128, 2, 1024], FP8, tag=f"expT{kp}_{nh}",
                      name=f"expT{kp}_{nh}")
             for nh in range(2)]
            for kp in range(KP)
        ]

        def scores_unit(km, nh):
            ps = ps_sc.tile([128, 1024], F32, tag="ps_sc")
            for half in range(2):
                nq = nh * 2 + half
                nc.tensor.matmul(
                    ps[:, half * 512:(half + 1) * 512],
                    lhsT=kt_sb[:, :, km * 128:(km + 1) * 128],
                    rhs=qt_sb[:, :, nq * 512:(nq + 1) * 512],
                    perf_mode=PM_DR,
                    start=True,
                    stop=True,
                )
            nc.scalar.activation(
                expT[km // 2][nh][:, km % 2, :], ps[:], AF.Exp, scale=SCALE
            )

        # ---------------- context + residual + out DMA ----------------------
        out_c = [stage.tile([128, 2 * D], F32, tag="outc", name=f"outc{c}")
                 for c in range(QB // 2)]
        ctxA = [big.tile([128, D + 1], BF16, tag=f"ctxA{i}", name=f"ctxA{i}")
                for i in range(8)]

        def ctx_mm(pc, qb, kp, start, stop):
            h, qq = qb // 8, qb % 8
            nc.tensor.matmul(
                pc,
                lhsT=expT[kp][h][:, :, qq * 128:(qq + 1) * 128],
                rhs=key2[kp][:],
                perf_mode=PM_DR,
                start=start,
                stop=stop,
            )

        def ctx_finish(pc, qb, div_act=False, add_dve=False):
            rc = small.tile([128, 1], F32, tag="recip")
            nc.vector.reciprocal(rc[:], pc[:, D:D + 1])
            c = qb // 2
            osl = out_c[c][:, (qb % 2) * D:(qb % 2 + 1) * D]
            if div_act:
                nc.scalar.activation(osl, pc[:, :D], AF.Identity, scale=rc[:])
            else:
                nc.vector.tensor_scalar(osl, pc[:, :D], rc[:], None, MULT)
            qg, j = qb // 4, qb % 4
            o2 = osl.rearrange("p (a b) -> p a b", a=2)
            if add_dve:
                nc.vector.tensor_tensor(o2, o2, qres3[qg][:, j, :, :], ADD)
            else:
                nc.gpsimd.tensor_add(o2, o2, qres3[qg][:, j, :, :])
            if qb % 2 == 1:
                nc.sync.dma_start(
                    out=out[c * 256:(c + 1) * 256, :].rearrange(
                        "(t p) d -> p t d", p=128),
                    in_=out_c[c][:].rearrange("p (t d) -> p t d", d=D),
                )

        def ctx_unit_h0(qb):
            pc_full = ps_a.tile([128, 512], F32, tag="ps_a")
            pc = pc_full[:, :D + 1]
            for kp in range(KP):
                ctx_mm(pc, qb, kp, kp == 0, kp == KP - 1)
            ctx_finish(pc, qb)

        def ctx_h1_A(qb):
            pc_full = ps_a.tile([128, 512], F32, tag="ps_a")
            pc = pc_full[:, :D + 1]
            for kp in range(KP_A):
                ctx_mm(pc, qb, kp, kp == 0, kp == KP_A - 1)
            nc.vector.tensor_copy(ctxA[qb - 8][:], pc)

        def ctx_h1_B(qb):
            pc_full = ps_a.tile([128, 512], F32, tag="ps_a")
            pc = pc_full[:, :D + 1]
            for kp in range(KP_A, KP):
                ctx_mm(pc, qb, kp, kp == KP_A, kp == KP - 1)
            nc.vector.tensor_tensor(pc, pc, ctxA[qb - 8][:], ADD)
            ctx_finish(pc, qb, div_act=True, add_dve=(qb % 4 >= 2))

        # ================= emission schedule ================================
        def units(fn, idxs):
            return [lambda i=i: fn(*i) if isinstance(i, tuple) else fn(i)
                    for i in idxs]

        def interleave(a, b, ratio):
            a = list(a)
            b = list(b)
            ia = ib = 0
            credit = 0.0
            while ia < len(a) or ib < len(b):
                if ia < len(a):
                    a[ia]()
                    ia += 1
                credit += ratio
                while credit >= 1.0 and ib < len(b):
                    b[ib]()
                    ib += 1
                    credit -= 1.0
            while ib < len(b):
                b[ib]()
                ib += 1

        # --- phase 1: KT as d2T lands, then QT nq0/nq1 as chunks land ---
        for nk in range(4):
            for dc in range(2):
                kt_unit(dc, nk)
        # nq0/nq1: scores-path evict split ACT (dc0) / DVE (dc1) so both
        # finish ~0.6us after the chunk's last matmul
        for nq in range(2):
            qt_unit(0, nq, sc_act=True, rs_act=False)
            qt_unit(1, nq, sc_act=False, rs_act=True)
        for qg in range(2):
            for dc in range(2):
                qres_xbar(qg, dc)

        # --- phase 2: scores-h0 interleaved with QT nq2/3 + key transposes ---
        def qres_late(qg):
            for dc in range(2):
                qres_xbar(qg, dc)

        filler = (
            units(key_tr, [0, 1, 2, 3])
            + [lambda: qt_unit(0, 2, False, False),
               lambda: qt_unit(1, 2, False, False),
               lambda: qt_unit(0, 3, False, False),
               lambda: qt_unit(1, 3, False, False),
               lambda: qres_late(2), lambda: qres_late(3)]
            + units(key_tr, [4, 5, 6, 7])
        )
        scores_h0 = units(scores_unit, [(km, 0) for km in range(KB)])
        interleave(scores_h0, filler, len(filler) / len(scores_h0))

        # --- phase 3: scores-h1 with ctx-h0 and ctx-h1-A ---
        # ctx_h1_A reads exps km0-13 of h1, so A units may only be emitted
        # after scores-h1 km13 (Tile orders by emission-time dependencies)
        sc_h1_a = units(scores_unit, [(km, 1) for km in range(2 * KP_A)])
        sc_h1_b = units(scores_unit, [(km, 1) for km in range(2 * KP_A, KB)])
        ctx0 = units(ctx_unit_h0, list(range(0, 8)))
        ctxa = units(ctx_h1_A, list(range(8, 16)))
        interleave(sc_h1_a, ctx0, len(ctx0) / len(sc_h1_a))
        interleave(sc_h1_b, ctxa, len(ctxa) / len(sc_h1_b))

        # --- phase 4: ctx-h1-B tail ---
        for qb in range(8, 16):
            ctx_h1_B(qb)

    nc.compile()
    return nc


_NC = None
_last_in_maps = None


def make_host_inputs(data1_b, data2_b, Wq, bq, Wk, bk):
    """Pack one batch element's inputs into the device layout (bf16 + f32).

    d1t[n, p, j, q'] = data1[n*512 + q', 8p + j]   (q-chunked, i p-major)
    d2t[p, j, k]     = data2[k, 2p + j]
    wq[p, j*256+d]   = Wq[8p + j, d]; wk[p, j*256+d] = Wk[2p + j, d]
    """
    bf = ml_dtypes.bfloat16
    a1 = np.asarray(data1_b, np.float32).astype(bf)      # [2048, 1024]
    d1t = np.ascontiguousarray(
        a1.reshape(4, 512, 128, J1).transpose(0, 2, 3, 1))
    a2 = np.asarray(data2_b, np.float32).astype(bf)      # [2048, 256]
    d2t = np.ascontiguousarray(
        a2.reshape(LK, 128, J2).transpose(1, 2, 0))
    Wq = np.asarray(Wq, dtype=np.float32)
    Wk = np.asarray(Wk, dtype=np.float32)
    bq = np.asarray(bq, dtype=np.float32)
    bk = np.asarray(bk, dtype=np.float32)
    wq = np.ascontiguousarray(
        Wq.astype(bf).reshape(128, J1, D).reshape(128, J1 * D))
    wk = np.ascontiguousarray(
        Wk.astype(bf).reshape(128, J2, D).reshape(128, J2 * D))
    bias = np.empty((128, 4), np.float32)
    bqk = bq + bk
    for c in range(2):
        bias[:, c] = bq[c * 128:(c + 1) * 128]
        bias[:, 2 + c] = bqk[c * 128:(c + 1) * 128]
    return {"d1t": d1t, "d2t": d2t, "wq": wq, "wk": wk, "bias": bias}


def _get_nc():
    global _NC
    if _NC is None:
        _NC = _build()
    return _NC


def kernel(data1, data2, Wq, bq, Wk, bk):
    global _last_in_maps
    data1 = np.asarray(data1, dtype=np.float32)
    data2 = np.asarray(data2, dtype=np.float32)

    nc = _get_nc()
    shared = None
    in_maps = []
    for b in range(B):
        m = make_host_inputs(data1[b], data2[b], Wq, bq, Wk, bk)
        if shared is None:
            shared = {k: m[k] for k in ("wq", "wk", "bias")}
        m.update(shared)
        in_maps.append(m)
    _last_in_maps = in_maps
    res = run_bass_kernel_spmd(nc, in_maps, core_ids=list(range(N_CORES)))
    return np.stack([res.results[i]["out"] for i in range(B)], axis=0)
